# revision 25
# baseline (speedup 1.0000x reference)
"""Hypernetwork causal attention (nn_Attention_87926570484382) on 8 TRN2 cores.

Strategy (single launch, batch-sharded attention, host-generated weights):
  host   : time-embedding MLP -> t [128]; W_attn/W_proj = t . fW (a 268-MFLOP
           matvec the host performs while casting fW anyway); bias assembly;
           1/sqrt(D) folded into q columns; x transposed per core.
  device : each core runs full attention for 2 of the 16 batches.  The PE
           instruction stream is interleaved so dense QKV/proj matmuls fill
           the gaps while the Scalar engine computes softmax exps (keeps the
           PE HAM clock-gate warm):
             qkv    : qkT[128, S] tiles (heads pairs on partitions), v into
                      ones-extended v_ext stationaries
             scores : row-tiled pairs - head h on partitions 0-63, head h+1
                      on 64-127 run concurrently on the PE array
             exp    : Scalar engine only (no table thrash)
             attnV  : v_ext stationary [64 v | 64 ones] so softmax
                      denominators accumulate for free in psum rows 64-127
             norm   : reciprocal_approx_fast + mul on Vector, direct from
                      PSUM
             proj   : K=128 matmuls; output bias added during the Vector
                      PSUM evacuation (no rank-1 bias matmuls)
"""

import os
import sys

import numpy as np

# ---------------------------------------------------------------------------
# Environment shims (must precede concourse imports in fresh environments)
# ---------------------------------------------------------------------------


def _ensure_axon_hooks():
    """Provide antenv.axon_hooks if the installed antenv lacks it (needed
    only when tracing; harmless otherwise)."""
    try:
        import antenv.axon_hooks  # noqa: F401
        return
    except ImportError:
        pass
    try:
        import antenv
    except ImportError:
        return
    import contextlib
    import ctypes
    import types

    mod = types.ModuleType("antenv.axon_hooks")
    mod._HOOK = None
    mod._TRIED = False

    def set_axon_ntff_profile_hook(hook):
        mod._HOOK = hook

    def _build(so_path):
        lib = ctypes.CDLL(so_path)
        if not hasattr(lib, "axon_start_nrt_profile"):
            return None
        lib.axon_start_nrt_profile.argtypes = [
            ctypes.POINTER(ctypes.c_int64),
            ctypes.c_size_t,
        ]
        lib.axon_start_nrt_profile.restype = ctypes.c_int64
        lib.axon_stop_nrt_profile.argtypes = [ctypes.c_char_p]
        lib.axon_stop_nrt_profile.restype = ctypes.c_int64

        @contextlib.contextmanager
        def _hook(output_dir, device_ids):
            import jax

            jax.devices()

            def _start():
                if device_ids:
                    ids = (ctypes.c_int64 * len(device_ids))(*device_ids)
                    return lib.axon_start_nrt_profile(ids, len(device_ids))
                return lib.axon_start_nrt_profile(None, 0)

            rc = _start()
            if rc != 0:
                # a stale profile session (crashed earlier run) blocks new
                # ones; stop it and retry once
                try:
                    lib.axon_stop_nrt_profile(str(output_dir).encode())
                except Exception:
                    pass
                rc = _start()
            started = rc == 0
            if not started:
                print(f"profile start failed rc={rc}; running untraced",
                      file=sys.stderr)
            try:
                yield
            finally:
                if started:
                    n = lib.axon_stop_nrt_profile(str(output_dir).encode())
                    print(f"profile: {n} file(s) -> {output_dir}",
                          file=sys.stderr)

        return _hook

    def get_axon_ntff_profile_hook():
        if mod._HOOK is None and not mod._TRIED:
            mod._TRIED = True
            p = "/opt/axon/libaxon_pjrt.so"
            if os.path.exists(p):
                try:
                    mod._HOOK = _build(p)
                except OSError:
                    mod._HOOK = None
        return mod._HOOK

    mod.set_axon_ntff_profile_hook = set_axon_ntff_profile_hook
    mod.get_axon_ntff_profile_hook = get_axon_ntff_profile_hook
    sys.modules["antenv.axon_hooks"] = mod
    antenv.axon_hooks = mod


_ensure_axon_hooks()

import concourse.bass as bass  # noqa: E402
import concourse.mybir as mybir  # noqa: E402
from concourse import tile as _tile  # noqa: E402
from concourse.tile import TileContext  # noqa: E402
from concourse.vector_clock import ScopedClock  # noqa: E402
from concourse.bass_utils import run_bass_kernel_spmd  # noqa: E402

F32 = mybir.dt.float32
F16 = mybir.dt.float16
FP8 = mybir.dt.float8e4
I32 = mybir.dt.int32
RECIP_MAGIC = float(0x7EF311C3)  # fast-inverse seed constant
WA_SCALE = 32.0                  # fp8 scale on Wa qk columns
EXP_SCALE = 1.0 / (WA_SCALE * WA_SCALE)

# problem constants (hardcoded per harness contract)
SIN_DIM, TEMBED = 64, 128
E, H, D = 512, 8, 64
B, S = 16, 1024
NCORES = 8
BPC = B // NCORES          # batches per core
S2 = BPC * S               # 2048 rows per core
J3 = 3 * H * D             # 1536
NET = E // 128             # 4 contraction tiles
NQT = S // 128             # 8 k/q tiles per batch

# ---------------------------------------------------------------------------
# Tile framework workarounds: this walrus accepts at most ONE semaphore wait
# and one update per instruction.
# ---------------------------------------------------------------------------

_NOP_CTR = [0]


def _patched_drain_and_barrier(self, tick_clock, wait_clock):
    carrier = self.nc.sync.nop(nofuse=True)
    wait_clock.add_sem_waits(
        carrier.ins, ScopedClock({None: tick_clock.global_clock})
    )
    si = carrier.ins.sync_info
    waits = list(si.on_wait) if si and si.on_wait else []
    if len(waits) > 1:
        carrier.ins.sync_info = mybir.SyncInfo(
            on_wait=waits[:1],
            on_update=list(si.on_update) if si and si.on_update else [],
        )
        for w in waits[1:]:
            extra = self.nc.sync.nop(nofuse=True)
            extra.ins.sync_info = mybir.SyncInfo(on_wait=[w], on_update=[])
    self.nc.sync.drain()
    self.nc.all_engine_barrier()
    assert self.sems is not None
    popped = self.nc._tile_sem_poison_stack.pop()
    assert popped is self._sem_poison
    self.nc.clear_and_free_semaphores(list(self.sems.allocated().values()))
    self.nc.all_engine_barrier()


_tile.TileContext._drain_and_barrier = _patched_drain_and_barrier


def _split_multi_waits(nc):
    for f in nc.m.functions:
        for blk in f.blocks:
            out = []
            changed = False
            for inst in blk.instructions:
                si = inst.sync_info
                waits = list(si.on_wait) if si and si.on_wait else []
                updates = list(si.on_update) if si and si.on_update else []
                is_dma = "DMA" in type(inst).__name__
                if len(waits) > 1:
                    changed = True
                    for w in waits[:-1]:
                        _NOP_CTR[0] += 1
                        nop = mybir.InstNoOp(
                            name=f"wsplit_{_NOP_CTR[0]}", ins=[], outs=[]
                        )
                        nop.engine = inst.engine
                        nop.sync_info = mybir.SyncInfo(on_wait=[w], on_update=[])
                        out.append(nop)
                    waits = [waits[-1]]
                    inst.sync_info = mybir.SyncInfo(
                        on_wait=waits, on_update=updates
                    )
                out.append(inst)
                if len(updates) > 1:
                    if is_dma:
                        raise AssertionError(
                            f"DMA {inst.name} has {len(updates)} updates"
                        )
                    changed = True
                    inst.sync_info = mybir.SyncInfo(
                        on_wait=waits, on_update=[updates[0]]
                    )
                    for u in updates[1:]:
                        _NOP_CTR[0] += 1
                        nop = mybir.InstNoOp(
                            name=f"usplit_{_NOP_CTR[0]}", ins=[], outs=[]
                        )
                        nop.engine = inst.engine
                        nop.sync_info = mybir.SyncInfo(on_wait=[], on_update=[u])
                        out.append(nop)
            if changed:
                blk.instructions = out
    return nc


# ---------------------------------------------------------------------------
# Single launch: attention for 2 batches per core
# ---------------------------------------------------------------------------


def _dedupe_ldweights(nc):
    """Replace Ldweights that reload the identical stationary into an
    untouched PE row-group range with NoOps (keeps sync_info, ~free)."""
    removed = 0
    for f in nc.m.functions:
        for blk in f.blocks:
            cur = {}  # row-group (0/64) -> key of resident weights
            for idx, inst in enumerate(blk.instructions):
                if getattr(inst, "engine", None) != mybir.EngineType.PE:
                    continue
                tname = type(inst).__name__
                if tname != "InstLdweights":
                    continue
                w = inst.ins[0]
                ap = w.value if hasattr(w, "value") else w
                try:
                    key = (
                        ap.memref,
                        ap.offset,
                        str(ap.ap),
                        str(getattr(ap, "dtype", None)),
                        str(getattr(inst, "perf_mode", None)),
                        str(getattr(inst, "tile_position", None)),
                    )
                    tp = getattr(inst, "tile_position", None) or (0, 0)
                    psz = ap.ap[0][1] if ap.ap else 128
                except Exception:
                    cur = {}
                    continue
                base = tp[0]
                groups = []
                if base < 64 and base + psz > 0:
                    groups.append(0)
                if base + psz > 64:
                    groups.append(64)
                if all(cur.get(g) == key for g in groups):
                    nop = mybir.InstNoOp(
                        name=f"ldwdedup_{removed}", ins=[], outs=[]
                    )
                    nop.engine = inst.engine
                    nop.sync_info = inst.sync_info
                    blk.instructions[idx] = nop
                    removed += 1
                else:
                    for g in groups:
                        cur[g] = key
    return removed


def build_attn(split_waits=True, dedupe=False, salt=None):
    # NOTE: the _dedupe_ldweights pass produced NaNs on hardware (the PE
    # appears to pair each Matmult with its own preceding Ldweights) -
    # keep it disabled.
    nc = bass.Bass()
    # fp8 qk-gen operands.  wa8: [128, (es=2, g=2, 1024 qk-cols)] with
    # E-feature 256*es + 128*g + p at partition p.  xt8: [128, (sc=4,
    # es=2, g=2, 512 s-cols)].
    wa8 = nc.dram_tensor("wa8", [128, 4096], FP8, kind="ExternalInput")
    xt8 = nc.dram_tensor("xt8", [128, 8192], FP8, kind="ExternalInput")
    xt = nc.dram_tensor("xt", [E, S2], F16, kind="ExternalInput")
    wav = nc.dram_tensor("wav", [E, 512], F16, kind="ExternalInput")
    wp = nc.dram_tensor("wp", [E, E], F16, kind="ExternalInput")
    bqk = nc.dram_tensor("bqk", [128, 8], F32, kind="ExternalInput")
    bcol = nc.dram_tensor("bcol", [128, 4], F32, kind="ExternalInput")
    mask2 = nc.dram_tensor("mask2", [128, 256], F16, kind="ExternalInput")
    vinit = nc.dram_tensor("vinit", [128, S], F16, kind="ExternalInput")
    out = nc.dram_tensor("out", [E, S2], F16, kind="ExternalOutput")

    EXP = mybir.ActivationFunctionType.Exp
    DR = mybir.MatmulPerfMode.DoubleRow

    with TileContext(nc) as tc:
        with (
            tc.tile_pool(name="cst", bufs=1) as cst,
            tc.tile_pool(name="qk", bufs=1) as qkp,
            tc.tile_pool(name="vx", bufs=1) as vxp,
            tc.tile_pool(name="ot", bufs=1) as otp,
            tc.tile_pool(name="ex", bufs=10) as exp_pool,
            tc.tile_pool(name="nr", bufs=6) as nrp,
            tc.tile_pool(name="ob", bufs=3) as obp,
            tc.tile_pool(name="ps", bufs=3, space="PSUM") as ps,
            tc.tile_pool(name="po", bufs=4, space="PSUM") as pop,
            tc.tile_pool(name="pq", bufs=1, space="PSUM") as pqp,
        ):
            # ---- PE warm-up: ~10 dummy matmuls on a zeroed scratch tile
            # keep the HAM activity monitor busy during the input-DMA wait
            # so the first real matmuls run at 2.4 GHz; a dummy exp pulls
            # the ACT table load off the critical path. ----
            dumw = cst.tile([128, 512], F16, tag="dumw", name="dumw")
            nc.gpsimd.memset(dumw[:, :], 0.0)
            warm_sb = cst.tile([64, 16], F16, tag="warmsb", name="warmsb")
            pqw = pqp.tile([128, 512], F32, tag="pq", name="pq_warm")
            for _ in range(10):
                nc.tensor.matmul(
                    pqw, dumw[:, 0:128], dumw[:, :], start=True, stop=True
                )
            nc.scalar.activation(
                warm_sb, dumw[0:64, 0:16],
                func=mybir.ActivationFunctionType.Exp,
            )

            # ---- input DMAs (priority order: qkv(b0) wants wa qk-cols +
            # first xt halves first) ----
            wa8_t = cst.tile([128, 4096], FP8, tag="wa8", name="wa8")
            xt8_t = cst.tile([128, 8192], FP8, tag="xt8", name="xt8")
            wavs = [
                cst.tile([128, 512], F16, tag=f"wav{et}", name=f"wav{et}")
                for et in range(NET)
            ]
            xts = [
                cst.tile([128, S2], F16, tag=f"xt{et}", name=f"xt{et}")
                for et in range(NET)
            ]
            wps = [
                cst.tile([128, E], F16, tag=f"wp{et}", name=f"wp{et}")
                for et in range(NET)
            ]
            # fp8 qk-gen operands first (the lead-in only needs these);
            # 2-way partition split halves the per-queue descriptor time
            for a in range(2):
                nc.sync.dma_start(
                    out=wa8_t[64 * a : 64 * (a + 1), :],
                    in_=wa8[64 * a : 64 * (a + 1), :],
                )
            for c in range(2):
                for a in range(2):
                    nc.sync.dma_start(
                        out=xt8_t[64 * a : 64 * (a + 1),
                                  2048 * c : 2048 * (c + 1)],
                        in_=xt8[64 * a : 64 * (a + 1),
                                2048 * c : 2048 * (c + 1)],
                    )
            bqk_t = cst.tile([128, 8], F32)
            nc.sync.dma_start(out=bqk_t, in_=bqk[:, :])
            mask2_t = cst.tile([128, 256], F16)
            nc.sync.dma_start(out=mask2_t, in_=mask2[:, :])
            for et in range(NET):
                nc.sync.dma_start(
                    out=xts[et][:, 0:1024],
                    in_=xt[128 * et : 128 * (et + 1), 0:1024],
                )
                nc.sync.dma_start(
                    out=wavs[et],
                    in_=wav[128 * et : 128 * (et + 1), :],
                )
            for c in range(2, 4):
                for a in range(2):
                    nc.sync.dma_start(
                        out=xt8_t[64 * a : 64 * (a + 1),
                                  2048 * c : 2048 * (c + 1)],
                        in_=xt8[64 * a : 64 * (a + 1),
                                2048 * c : 2048 * (c + 1)],
                    )
            for et in range(NET):
                nc.sync.dma_start(
                    out=xts[et][:, 1024:2048],
                    in_=xt[128 * et : 128 * (et + 1), 1024:2048],
                )
            for et in range(NET):
                nc.sync.dma_start(
                    out=wps[et], in_=wp[128 * et : 128 * (et + 1), :]
                )
            bcol_t = cst.tile([128, 4], F32)
            nc.sync.dma_start(out=bcol_t, in_=bcol[:, :])
            # persistent v_ext tiles: [128 k-rows, 8h * (64 v | 64 ones)];
            # ones columns are constant, v columns written by qkv v-units
            vxs = {}
            for b in range(BPC):
                for st in range(NQT):
                    v_ = vxp.tile(
                        [128, 8 * 128], F16, tag=f"vx{b}_{st}", name=f"vx{b}_{st}"
                    )
                    nc.sync.dma_start(out=v_, in_=vinit[:, :])
                    vxs[(b, st)] = v_

            qkts = {
                (b, m): qkp.tile(
                    [128, S], F16, tag=f"qk{b}_{m}", name=f"qkT{b}_{m}"
                )
                for b in range(BPC)
                for m in range(8)
            }
            ots = {
                (b, i): otp.tile(
                    [128, S], F16, tag=f"ot{b}_{i}", name=f"oT{b}_{i}"
                )
                for b in range(BPC)
                for i in range(4)
            }

            # ---- work units (emitted inline between attention steps so
            # dense PE work fills Scalar-engine stalls) ----
            emitted = set()

            wa8_4d = wa8_t[:, :].rearrange("p (e g j) -> p e g j", e=2, g=2)
            xt8_5d = xt8_t[:, :].rearrange(
                "p (c e g s) -> p c e g s", c=4, e=2, g=2
            )

            def qk_unit(b, m, sc, lead=False):
                """qkT[m] seq-chunk sc for batch b (+ bias).  Lead-in
                units allocate from the (idle at start) scores pool so the
                four units pipeline instead of serializing through the
                single pq bank, and evacuate on Scalar only."""
                pool = ps if lead else pqp
                tag = "ps" if lead else "pq"
                pq0 = pool.tile(
                    [128, 512], F32, tag=tag, name=f"pq_{b}_{m}_{sc}"
                )
                for es in range(2):
                    nc.tensor.matmul(
                        pq0,
                        wa8_4d[:, es, :, 128 * m : 128 * (m + 1)],
                        xt8_5d[:, 2 * b + sc, es, :, :],
                        start=(es == 0), stop=(es == 1),
                        perf_mode=DR,
                    )
                # alternate evacuation engine so the single pq psum buffer
                # recycles quickly (Identity+bias is in the exp table set,
                # so Scalar use causes no table thrash)
                if sc == 0 or lead:
                    nc.scalar.activation(
                        qkts[(b, m)][:, 512 * sc : 512 * (sc + 1)], pq0,
                        func=mybir.ActivationFunctionType.Identity,
                        bias=bqk_t[:, m : m + 1],
                    )
                else:
                    nc.vector.tensor_scalar_add(
                        qkts[(b, m)][:, 512:1024], pq0, bqk_t[:, m : m + 1]
                    )
                emitted.add(("qk", b, m, sc))

            def v_unit(b, st):
                """v rows for seq-tile st of batch b -> v_ext v-columns."""
                pv_ = pqp.tile([128, 512], F32, tag="pq", name=f"pv_{b}_{st}")
                s0 = b * S + 128 * st
                for et in range(NET):
                    nc.tensor.matmul(
                        pv_,
                        xts[et][:, s0 : s0 + 128],
                        wavs[et][:, :],
                        start=(et == 0),
                        stop=(et == NET - 1),
                    )
                dst = vxs[(b, st)][:, :].rearrange("p (h c) -> p h c", h=H)
                dst = dst[:, :, 0:64]
                src = pv_[:, :].rearrange("p (h c) -> p h c", h=H)
                nc.vector.tensor_copy(dst, src)
                emitted.add(("v", b, st))

            def proj_unit(b, eb, sc, alt_pool=False, evac_act=False):
                """output e-rows [128*eb:...] for seq-chunk sc of batch b
                (out is [E, S2]).  Tail units alternate psum pools so they
                pipeline instead of serializing through one bank."""
                pool = ps if alt_pool else pqp
                tag = "ps" if alt_pool else "pq"
                pp_ = pool.tile(
                    [128, 512], F32, tag=tag, name=f"pp_{b}_{eb}_{sc}"
                )
                for i in range(4):
                    nc.tensor.matmul(
                        pp_,
                        wps[i][:, 128 * eb : 128 * (eb + 1)],
                        ots[(b, i)][:, 512 * sc : 512 * (sc + 1)],
                        start=(i == 0), stop=(i == 3),
                    )
                ob_ = obp.tile(
                    [128, 512], F16, tag="ob", name=f"ob_{b}_{eb}_{sc}"
                )
                if sc == 0 or evac_act:
                    nc.scalar.activation(
                        ob_, pp_,
                        func=mybir.ActivationFunctionType.Identity,
                        bias=bcol_t[:, eb : eb + 1],
                    )
                else:
                    nc.vector.tensor_scalar_add(
                        ob_, pp_, bcol_t[:, eb : eb + 1]
                    )
                s0 = b * S + 512 * sc
                nc.sync.dma_start(
                    out=out[128 * eb : 128 * (eb + 1), s0 : s0 + 512],
                    in_=ob_,
                )

            def attn_pair(b, i, bg, slots=None):
                """Causal attention for heads (2i, 2i+1) of batch b.
                bg: list of background thunks consumed inside the j-loop."""
                for mm in (i, 4 + i):
                    for sc in range(2):
                        assert ("qk", b, mm, sc) in emitted
                kt = qkts[(b, 4 + i)]
                qt = qkts[(b, i)]
                # per-j exp tile holding BOTH heads: head hh at cols
                # hh*1024 + q (one mask instruction covers the pair)
                exs = {
                    j: exp_pool.tile(
                        [128, 2 * S], F16, tag="ex", name=f"ex_{b}_{i}_{j}"
                    )
                    for j in range(NQT)
                }

                bg = list(bg)
                nbg = len(bg)
                if slots is None:
                    # distribute bg thunks across the 8 j-iterations
                    slots = [nbg // NQT + (1 if j < nbg % NQT else 0)
                             for j in range(NQT)]
                slots = list(slots)
                assert sum(slots) == nbg
                bi = 0

                # qc1 accumulators live across the whole j-loop so their
                # matmuls can issue right after each k-block's exp (no
                # j==7 burst waiting on the Scalar engine)
                po1 = {
                    hh: pop.tile(
                        [128, 512], F32, tag="po", name=f"po1_{b}_{i}_{hh}"
                    )
                    for hh in range(2)
                }

                def attnv_qc1_step(hh, jj):
                    h = 2 * i + hh
                    c0 = max(512, 128 * jj)
                    nc.tensor.matmul(
                        po1[hh][:, c0 - 512 : 512],
                        vxs[(b, jj)][:, 128 * h : 128 * (h + 1)],
                        exs[jj][:, 1024 * hh + c0 : 1024 * hh + 1024],
                        start=(jj == 0),
                        stop=(jj == NQT - 1),
                    )

                def attn_pass(hh, qc):
                    h = 2 * i + hh
                    if qc == 1:
                        po_ = po1[hh]
                    else:
                        po_ = pop.tile(
                            [128, 512], F32, tag="po",
                            name=f"po_{b}_{i}_{hh}_{qc}",
                        )
                        for jj in range(4):
                            c0 = 128 * jj
                            nc.tensor.matmul(
                                po_[:, c0:512],
                                vxs[(b, jj)][:, 128 * h : 128 * (h + 1)],
                                exs[jj][:, 1024 * hh + c0 : 1024 * hh + 512],
                                start=(jj == 0),
                                stop=(jj == 3),
                            )
                    # 1/denominator via bit-trick seed + one Newton step
                    # (rel err ~2.6e-3, HW-verified); denominator copies sit
                    # in po rows 64-127 thanks to the ones-extended v_ext
                    sfx = f"{b}_{i}_{hh}_{qc}"
                    s_ = nrp.tile([64, 512], F32, tag="nr", name=f"nrs_{sfx}")
                    t_ = nrp.tile([64, 512], F32, tag="nr", name=f"nrt_{sfx}")
                    u_ = nrp.tile([64, 512], F32, tag="nr", name=f"nru_{sfx}")
                    nc.vector.tensor_scalar(
                        out=s_[:, :].bitcast(I32),
                        in0=po_[64:128, :].bitcast(I32),
                        scalar1=RECIP_MAGIC,
                        scalar2=-1.0,
                        op0=mybir.AluOpType.subtract,
                        op1=mybir.AluOpType.mult,
                    )
                    nc.vector.tensor_mul(t_, po_[64:128, :], s_)
                    # (GpSimd cannot take this step: [64,512] f32 ops on the
                    # Q7s measured ~4-8x slower and serialized the kernel)
                    nc.vector.scalar_tensor_tensor(
                        out=u_, in0=t_, scalar=2.0, in1=s_,
                        op0=mybir.AluOpType.subtract,
                        op1=mybir.AluOpType.mult,
                    )
                    nc.vector.scalar_tensor_tensor(
                        out=ots[(b, i)][64 * hh : 64 * hh + 64,
                                        512 * qc : 512 * (qc + 1)],
                        in0=po_[0:64, :], scalar=-1.0, in1=u_,
                        op0=mybir.AluOpType.mult,
                        op1=mybir.AluOpType.mult,
                    )

                for j in range(NQT):
                    nqc = sum(
                        1 for qc in range(2)
                        if max(512 * qc, 128 * j) < 512 * (qc + 1)
                    )
                    for qc in range(2):
                        c0 = max(512 * qc, 128 * j)
                        c1 = 512 * (qc + 1)
                        if c0 >= c1:
                            continue
                        w = c1 - c0
                        pA = ps.tile(
                            [128, 512], F32, tag="ps", name=f"sA_{b}_{i}_{j}_{qc}"
                        )
                        pB = ps.tile(
                            [128, 512], F32, tag="ps", name=f"sB_{b}_{i}_{j}_{qc}"
                        )
                        nc.tensor.matmul(
                            pA[:, 0:w],
                            kt[0:64, 128 * j : 128 * (j + 1)],
                            qt[0:64, c0:c1],
                            start=True,
                            stop=True,
                        )
                        nc.tensor.matmul(
                            pB[:, 0:w],
                            kt[64:128, 128 * j : 128 * (j + 1)],
                            qt[64:128, c0:c1],
                            start=True,
                            stop=True,
                        )
                        nc.scalar.activation(
                            exs[j][:, c0:c1], pA[:, 0:w],
                            func=EXP, scale=EXP_SCALE,
                        )
                        nc.scalar.activation(
                            exs[j][:, 1024 + c0 : 1024 + c1],
                            pB[:, 0:w],
                            func=EXP, scale=EXP_SCALE,
                        )
                        # keep the PE fed between the two psum-chunk
                        # rounds (the ps pool is fully occupied here)
                        if nqc == 2 and qc == 0 and bi < nbg and slots[j] > 0:
                            bg[bi]()
                            bi += 1
                            slots[j] -= 1
                    # causal mask on the diagonal 128-col blocks (GpSimd:
                    # SBUF-only f16 work; 2D APs only - 3D APs are ~6x
                    # slower on the Q7s)
                    nc.gpsimd.tensor_mul(
                        exs[j][:, 128 * j : 128 * (j + 1)],
                        exs[j][:, 128 * j : 128 * (j + 1)],
                        mask2_t[:, 0:128],
                    )
                    nc.gpsimd.tensor_mul(
                        exs[j][:, 1024 + 128 * j : 1024 + 128 * (j + 1)],
                        exs[j][:, 1024 + 128 * j : 1024 + 128 * (j + 1)],
                        mask2_t[:, 0:128],
                    )
                    # stream this k-block into the qc1 accumulators now
                    # that the diagonal is masked
                    attnv_qc1_step(0, j)
                    attnv_qc1_step(1, j)
                    for _ in range(slots[j]):
                        bg[bi]()
                        bi += 1
                    if j == 3:
                        attn_pass(0, 0)
                        attn_pass(1, 0)
                    if j == NQT - 1:
                        attn_pass(0, 1)
                        attn_pass(1, 1)
                assert bi == nbg

            # ---- schedule ----
            # lead-in: qk tiles for pair (0,0)
            for m in (0, 4):
                for sc in range(2):
                    qk_unit(0, m, sc, lead=True)

            def mkqk(b, m, sc):
                return lambda: qk_unit(b, m, sc)

            def mkv(b, st):
                return lambda: v_unit(b, st)

            def mkp(b, eb, sc):
                return lambda: proj_unit(b, eb, sc)

            def qks(b, m):
                return [mkqk(b, m, 0), mkqk(b, m, 1)]

            bg_for_pair = {
                (0, 0): [mkv(0, 0), mkv(0, 1), mkv(0, 2), mkv(0, 3),
                         mkqk(0, 1, 0), mkqk(0, 5, 0),
                         mkv(0, 4), mkv(0, 5), mkv(0, 6), mkv(0, 7),
                         mkqk(0, 1, 1), mkqk(0, 5, 1)],
                (0, 1): [mkqk(0, 2, 0), mkqk(0, 2, 1), mkv(1, 0),
                         mkqk(0, 6, 0), mkqk(0, 6, 1), mkv(1, 1)],
                (0, 2): [mkqk(0, 3, 0), mkqk(0, 3, 1), mkv(1, 2),
                         mkqk(0, 7, 0), mkqk(0, 7, 1), mkv(1, 3)],
                (0, 3): [mkqk(1, 0, 0), mkqk(1, 0, 1), mkv(1, 4),
                         mkqk(1, 4, 0), mkqk(1, 4, 1), mkv(1, 5)],
                (1, 0): [mkv(1, 6), mkv(1, 7), mkqk(1, 1, 0),
                         mkqk(1, 5, 0), mkqk(1, 1, 1), mkqk(1, 5, 1)],
                (1, 1): qks(1, 2) + qks(1, 6),
                (1, 2): qks(1, 3) + qks(1, 7)
                + [mkp(0, 0, 0), mkp(0, 0, 1), mkp(0, 1, 0), mkp(0, 1, 1)],
                # proj(1, eb, sc0) needs only the qc0 norms of batch 1 -
                # its own pair's land at j==3, so slots 4..7 are safe
                (1, 3): [mkp(0, 2, 0), mkp(0, 2, 1),
                         mkp(0, 3, 0), mkp(0, 3, 1),
                         mkp(1, 0, 0), mkp(1, 1, 0),
                         mkp(1, 2, 0), mkp(1, 3, 0)],
            }
            slots_override = {(1, 3): [1, 1, 1, 1, 1, 1, 1, 1]}
            for b in range(BPC):
                for i in range(4):
                    attn_pair(
                        b, i, bg_for_pair[(b, i)],
                        slots=slots_override.get((b, i)),
                    )
            # tail: batch-1 projection second halves (alternating psum
            # pools so evacuations overlap the next unit's matmuls)
            for eb in range(4):
                proj_unit(1, eb, 1, alt_pool=(eb % 2 == 1), evac_act=True)
    if salt:
        # unique NoOp name changes the BIR bytes -> defeats NEFF caching
        # when only compiler flags change
        nop = mybir.InstNoOp(name=f"salt_{salt}", ins=[], outs=[])
        nop.engine = mybir.EngineType.SP
        nop.sync_info = mybir.SyncInfo(on_wait=[], on_update=[])
        nc.m.functions[0].blocks[0].instructions.append(nop)
    if dedupe:
        _dedupe_ldweights(nc)
    if split_waits:
        _split_multi_waits(nc)
    return nc


# ---------------------------------------------------------------------------
# Host orchestration
# ---------------------------------------------------------------------------

_CACHE = {}


def _get(name, builder):
    if name not in _CACHE:
        _CACHE[name] = builder()
    return _CACHE[name]


def _run_with_retry(nc, in_maps, trace=False, tries=3):
    import time as _time

    last = None
    for attempt in range(tries):
        try:
            return run_bass_kernel_spmd(
                nc, in_maps, core_ids=list(range(NCORES)), trace=trace
            )
        except Exception as e:  # transient NRT_EXEC_UNIT_UNRECOVERABLE etc.
            last = e
            _time.sleep(2.0 * (attempt + 1))
    raise last


def _silu(v):
    return v / (1.0 + np.exp(-v))


def kernel(
    time_embed,
    x,
    lin1_w,
    lin1_b,
    lin2_w,
    lin2_b,
    fW_attn_w,
    fW_attn_b,
    fb_attn,
    fW_proj_w,
    fW_proj_b,
    fb_proj,
    _trace=False,
    _times=None,
):
    f64 = np.float64
    # ---- host: time-embedding MLP ----
    t1 = _silu(time_embed.astype(f64) @ lin1_w.astype(f64) + lin1_b.astype(f64))
    t = t1 @ lin2_w.astype(f64) + lin2_b.astype(f64)   # [128]
    t32 = t.astype(np.float32)

    # ---- host: hypernetwork weights (the host reads fW to cast anyway;
    # this matvec is 0.4% of the problem's FLOPs) ----
    Wa = t32 @ fW_attn_w.reshape(TEMBED, E * J3)
    Wa = Wa.reshape(E, J3) + fW_attn_b.reshape(E, J3)
    Wp = t32 @ fW_proj_w.reshape(TEMBED, E * E)
    Wp = Wp.reshape(E, E) + fW_proj_b.reshape(E, E)
    Wa[:, :512] *= 0.125  # fold 1/sqrt(D) into q columns

    # ---- host: fp8 qk-gen stationary: [128, (es, g, 1024 qk-cols)] ----
    fp8np = mybir.dt.np(FP8)
    Wa_qk = (Wa[:, :1024] * WA_SCALE).astype(fp8np)
    wa8_in = np.ascontiguousarray(
        Wa_qk.reshape(2, 2, 128, 1024).transpose(2, 0, 1, 3).reshape(128, 4096)
    )

    # ---- host: biases ----
    b_attn = (t @ fb_attn.astype(f64).reshape(TEMBED, J3)).astype(np.float32)
    bqk_host = b_attn[:1024].copy()
    bqk_host[:512] *= 0.125
    bqk_host *= WA_SCALE  # qkT tiles are stored x32
    bqk_in = np.ascontiguousarray(bqk_host.reshape(8, 128).T)
    b_v = b_attn[1024:]
    b_proj = (t @ fb_proj.astype(f64)).astype(np.float32)
    brow = (b_v.astype(f64) @ Wp.astype(f64) + b_proj).astype(np.float32)
    bcol_in = np.ascontiguousarray(brow.reshape(4, 128).T)
    mask1 = np.triu(np.ones((128, 128), dtype=np.float16))
    mask2_in = np.ascontiguousarray(np.concatenate([mask1, mask1], axis=1))
    vinit_in = np.zeros((128, S), dtype=np.float16)
    for h in range(H):
        vinit_in[:, 128 * h + 64 : 128 * (h + 1)] = 1.0
    Wav16 = np.ascontiguousarray(Wa[:, 1024:1536].astype(np.float16))
    Wp16 = Wp.astype(np.float16)

    # ---- launch: attention ----
    nc_attn = _get("attn", build_attn)
    in_maps = []
    for c in range(NCORES):
        xt_c = np.ascontiguousarray(
            x[BPC * c : BPC * (c + 1)].reshape(S2, E).T
        )
        xt8_c = np.ascontiguousarray(
            xt_c.astype(fp8np)
            .reshape(2, 2, 128, 4, 512)     # [es][g][p][sc][sl]
            .transpose(2, 3, 0, 1, 4)       # [p][sc][es][g][sl]
            .reshape(128, 8192)
        )
        in_maps.append(
            {
                "wa8": wa8_in,
                "xt8": xt8_c,
                "xt": xt_c.astype(np.float16),
                "wav": Wav16,
                "wp": Wp16,
                "bqk": bqk_in,
                "bcol": bcol_in,
                "mask2": mask2_in,
                "vinit": vinit_in,
            }
        )
    res = _run_with_retry(nc_attn, in_maps, trace=_trace)
    if _times is not None:
        _times.append(res.exec_time_ns)

    out = np.empty((B, S, E), dtype=np.float32)
    for c in range(NCORES):
        out[BPC * c : BPC * (c + 1)] = (
            res.results[c]["out"].astype(np.float32).T.reshape(BPC, S, E)
        )
    return out



# revision 27
# speedup vs baseline: 1.0315x; 1.0315x over previous
"""Hypernetwork causal attention (nn_Attention_87926570484382) on 8 TRN2 cores.

Strategy (single launch, batch-sharded attention, host-generated weights):
  host   : time-embedding MLP -> t [128]; W_attn/W_proj = t . fW (a 268-MFLOP
           matvec the host performs while casting fW anyway); bias assembly;
           1/sqrt(D) folded into q columns; x transposed per core.
  device : each core runs full attention for 2 of the 16 batches.  The PE
           instruction stream is interleaved so dense QKV/proj matmuls fill
           the gaps while the Scalar engine computes softmax exps (keeps the
           PE HAM clock-gate warm):
             qkv    : qkT[128, S] tiles (heads pairs on partitions), v into
                      ones-extended v_ext stationaries
             scores : row-tiled pairs - head h on partitions 0-63, head h+1
                      on 64-127 run concurrently on the PE array
             exp    : Scalar engine only (no table thrash)
             attnV  : v_ext stationary [64 v | 64 ones] so softmax
                      denominators accumulate for free in psum rows 64-127
             norm   : reciprocal_approx_fast + mul on Vector, direct from
                      PSUM
             proj   : K=128 matmuls; output bias added during the Vector
                      PSUM evacuation (no rank-1 bias matmuls)
"""

import os
import sys

import numpy as np

# ---------------------------------------------------------------------------
# Environment shims (must precede concourse imports in fresh environments)
# ---------------------------------------------------------------------------


def _ensure_axon_hooks():
    """Provide antenv.axon_hooks if the installed antenv lacks it (needed
    only when tracing; harmless otherwise)."""
    try:
        import antenv.axon_hooks  # noqa: F401
        return
    except ImportError:
        pass
    try:
        import antenv
    except ImportError:
        return
    import contextlib
    import ctypes
    import types

    mod = types.ModuleType("antenv.axon_hooks")
    mod._HOOK = None
    mod._TRIED = False

    def set_axon_ntff_profile_hook(hook):
        mod._HOOK = hook

    def _build(so_path):
        lib = ctypes.CDLL(so_path)
        if not hasattr(lib, "axon_start_nrt_profile"):
            return None
        lib.axon_start_nrt_profile.argtypes = [
            ctypes.POINTER(ctypes.c_int64),
            ctypes.c_size_t,
        ]
        lib.axon_start_nrt_profile.restype = ctypes.c_int64
        lib.axon_stop_nrt_profile.argtypes = [ctypes.c_char_p]
        lib.axon_stop_nrt_profile.restype = ctypes.c_int64

        @contextlib.contextmanager
        def _hook(output_dir, device_ids):
            import jax

            jax.devices()

            def _start():
                if device_ids:
                    ids = (ctypes.c_int64 * len(device_ids))(*device_ids)
                    return lib.axon_start_nrt_profile(ids, len(device_ids))
                return lib.axon_start_nrt_profile(None, 0)

            rc = _start()
            if rc != 0:
                # a stale profile session (crashed earlier run) blocks new
                # ones; stop it and retry once
                try:
                    lib.axon_stop_nrt_profile(str(output_dir).encode())
                except Exception:
                    pass
                rc = _start()
            started = rc == 0
            if not started:
                print(f"profile start failed rc={rc}; running untraced",
                      file=sys.stderr)
            try:
                yield
            finally:
                if started:
                    n = lib.axon_stop_nrt_profile(str(output_dir).encode())
                    print(f"profile: {n} file(s) -> {output_dir}",
                          file=sys.stderr)

        return _hook

    def get_axon_ntff_profile_hook():
        if mod._HOOK is None and not mod._TRIED:
            mod._TRIED = True
            p = "/opt/axon/libaxon_pjrt.so"
            if os.path.exists(p):
                try:
                    mod._HOOK = _build(p)
                except OSError:
                    mod._HOOK = None
        return mod._HOOK

    mod.set_axon_ntff_profile_hook = set_axon_ntff_profile_hook
    mod.get_axon_ntff_profile_hook = get_axon_ntff_profile_hook
    sys.modules["antenv.axon_hooks"] = mod
    antenv.axon_hooks = mod


_ensure_axon_hooks()

import concourse.bass as bass  # noqa: E402
import concourse.mybir as mybir  # noqa: E402
from concourse import tile as _tile  # noqa: E402
from concourse.tile import TileContext  # noqa: E402
from concourse.vector_clock import ScopedClock  # noqa: E402
from concourse.bass_utils import run_bass_kernel_spmd  # noqa: E402

F32 = mybir.dt.float32
F16 = mybir.dt.float16
FP8 = mybir.dt.float8e4
I32 = mybir.dt.int32
RECIP_MAGIC = float(0x7EF311C3)  # fast-inverse seed constant
WA_SCALE = 32.0                  # fp8 scale on Wa qk columns
EXP_SCALE = 1.0 / (WA_SCALE * WA_SCALE)

# problem constants (hardcoded per harness contract)
SIN_DIM, TEMBED = 64, 128
E, H, D = 512, 8, 64
B, S = 16, 1024
NCORES = 8
BPC = B // NCORES          # batches per core
S2 = BPC * S               # 2048 rows per core
J3 = 3 * H * D             # 1536
NET = E // 128             # 4 contraction tiles
NQT = S // 128             # 8 k/q tiles per batch

# ---------------------------------------------------------------------------
# Tile framework workarounds: this walrus accepts at most ONE semaphore wait
# and one update per instruction.
# ---------------------------------------------------------------------------

_NOP_CTR = [0]


def _patched_drain_and_barrier(self, tick_clock, wait_clock):
    carrier = self.nc.sync.nop(nofuse=True)
    wait_clock.add_sem_waits(
        carrier.ins, ScopedClock({None: tick_clock.global_clock})
    )
    si = carrier.ins.sync_info
    waits = list(si.on_wait) if si and si.on_wait else []
    if len(waits) > 1:
        carrier.ins.sync_info = mybir.SyncInfo(
            on_wait=waits[:1],
            on_update=list(si.on_update) if si and si.on_update else [],
        )
        for w in waits[1:]:
            extra = self.nc.sync.nop(nofuse=True)
            extra.ins.sync_info = mybir.SyncInfo(on_wait=[w], on_update=[])
    self.nc.sync.drain()
    self.nc.all_engine_barrier()
    assert self.sems is not None
    popped = self.nc._tile_sem_poison_stack.pop()
    assert popped is self._sem_poison
    self.nc.clear_and_free_semaphores(list(self.sems.allocated().values()))
    self.nc.all_engine_barrier()


_tile.TileContext._drain_and_barrier = _patched_drain_and_barrier


def _split_multi_waits(nc):
    for f in nc.m.functions:
        for blk in f.blocks:
            out = []
            changed = False
            for inst in blk.instructions:
                si = inst.sync_info
                waits = list(si.on_wait) if si and si.on_wait else []
                updates = list(si.on_update) if si and si.on_update else []
                is_dma = "DMA" in type(inst).__name__
                if len(waits) > 1:
                    changed = True
                    for w in waits[:-1]:
                        _NOP_CTR[0] += 1
                        nop = mybir.InstNoOp(
                            name=f"wsplit_{_NOP_CTR[0]}", ins=[], outs=[]
                        )
                        nop.engine = inst.engine
                        nop.sync_info = mybir.SyncInfo(on_wait=[w], on_update=[])
                        out.append(nop)
                    waits = [waits[-1]]
                    inst.sync_info = mybir.SyncInfo(
                        on_wait=waits, on_update=updates
                    )
                out.append(inst)
                if len(updates) > 1:
                    if is_dma:
                        raise AssertionError(
                            f"DMA {inst.name} has {len(updates)} updates"
                        )
                    changed = True
                    inst.sync_info = mybir.SyncInfo(
                        on_wait=waits, on_update=[updates[0]]
                    )
                    for u in updates[1:]:
                        _NOP_CTR[0] += 1
                        nop = mybir.InstNoOp(
                            name=f"usplit_{_NOP_CTR[0]}", ins=[], outs=[]
                        )
                        nop.engine = inst.engine
                        nop.sync_info = mybir.SyncInfo(on_wait=[], on_update=[u])
                        out.append(nop)
            if changed:
                blk.instructions = out
    return nc


# ---------------------------------------------------------------------------
# Single launch: attention for 2 batches per core
# ---------------------------------------------------------------------------


def _dedupe_ldweights(nc):
    """Replace Ldweights that reload the identical stationary into an
    untouched PE row-group range with NoOps (keeps sync_info, ~free)."""
    removed = 0
    for f in nc.m.functions:
        for blk in f.blocks:
            cur = {}  # row-group (0/64) -> key of resident weights
            for idx, inst in enumerate(blk.instructions):
                if getattr(inst, "engine", None) != mybir.EngineType.PE:
                    continue
                tname = type(inst).__name__
                if tname != "InstLdweights":
                    continue
                w = inst.ins[0]
                ap = w.value if hasattr(w, "value") else w
                try:
                    key = (
                        ap.memref,
                        ap.offset,
                        str(ap.ap),
                        str(getattr(ap, "dtype", None)),
                        str(getattr(inst, "perf_mode", None)),
                        str(getattr(inst, "tile_position", None)),
                    )
                    tp = getattr(inst, "tile_position", None) or (0, 0)
                    psz = ap.ap[0][1] if ap.ap else 128
                except Exception:
                    cur = {}
                    continue
                base = tp[0]
                groups = []
                if base < 64 and base + psz > 0:
                    groups.append(0)
                if base + psz > 64:
                    groups.append(64)
                if all(cur.get(g) == key for g in groups):
                    nop = mybir.InstNoOp(
                        name=f"ldwdedup_{removed}", ins=[], outs=[]
                    )
                    nop.engine = inst.engine
                    nop.sync_info = inst.sync_info
                    blk.instructions[idx] = nop
                    removed += 1
                else:
                    for g in groups:
                        cur[g] = key
    return removed


def build_attn(split_waits=True, dedupe=False, salt=None):
    # NOTE: the _dedupe_ldweights pass produced NaNs on hardware (the PE
    # appears to pair each Matmult with its own preceding Ldweights) -
    # keep it disabled.
    nc = bass.Bass()
    # fp8 qk-gen operands.  wa8: [128, (es=2, g=2, 1024 qk-cols)] with
    # E-feature 256*es + 128*g + p at partition p.  xt8: [128, (sc=4,
    # es=2, g=2, 512 s-cols)].
    wa8 = nc.dram_tensor("wa8", [128, 4096], FP8, kind="ExternalInput")
    xt8 = nc.dram_tensor("xt8", [128, 8192], FP8, kind="ExternalInput")
    xt = nc.dram_tensor("xt", [E, S2], F16, kind="ExternalInput")
    wav = nc.dram_tensor("wav", [E, 512], F16, kind="ExternalInput")
    wp = nc.dram_tensor("wp", [E, E], F16, kind="ExternalInput")
    bqk = nc.dram_tensor("bqk", [128, 8], F32, kind="ExternalInput")
    bcol = nc.dram_tensor("bcol", [128, 4], F32, kind="ExternalInput")
    mask2 = nc.dram_tensor("mask2", [128, 256], F16, kind="ExternalInput")
    vinit = nc.dram_tensor("vinit", [128, S], F16, kind="ExternalInput")
    out = nc.dram_tensor("out", [E, S2], F16, kind="ExternalOutput")

    EXP = mybir.ActivationFunctionType.Exp
    DR = mybir.MatmulPerfMode.DoubleRow

    with TileContext(nc) as tc:
        with (
            tc.tile_pool(name="cst", bufs=1) as cst,
            tc.tile_pool(name="qk", bufs=1) as qkp,
            tc.tile_pool(name="vx", bufs=1) as vxp,
            tc.tile_pool(name="ot", bufs=1) as otp,
            tc.tile_pool(name="ex", bufs=10) as exp_pool,
            tc.tile_pool(name="nr", bufs=6) as nrp,
            tc.tile_pool(name="ob", bufs=3) as obp,
            tc.tile_pool(name="ps", bufs=3, space="PSUM") as ps,
            tc.tile_pool(name="po", bufs=4, space="PSUM") as pop,
            tc.tile_pool(name="pq", bufs=1, space="PSUM") as pqp,
        ):
            # ---- PE warm-up: ~10 dummy matmuls on a zeroed scratch tile
            # keep the HAM activity monitor busy during the input-DMA wait
            # so the first real matmuls run at 2.4 GHz; a dummy exp pulls
            # the ACT table load off the critical path. ----
            dumw = cst.tile([128, 512], F16, tag="dumw", name="dumw")
            nc.gpsimd.memset(dumw[:, :], 0.0)
            warm_sb = cst.tile([64, 16], F16, tag="warmsb", name="warmsb")
            pqw = pqp.tile([128, 512], F32, tag="pq", name="pq_warm")
            for _ in range(10):
                nc.tensor.matmul(
                    pqw, dumw[:, 0:128], dumw[:, :], start=True, stop=True
                )
            nc.scalar.activation(
                warm_sb, dumw[0:64, 0:16],
                func=mybir.ActivationFunctionType.Exp,
            )

            # ---- input DMAs (priority order: qkv(b0) wants wa qk-cols +
            # first xt halves first) ----
            wa8_t = cst.tile([128, 4096], FP8, tag="wa8", name="wa8")
            xt8_t = cst.tile([128, 8192], FP8, tag="xt8", name="xt8")
            wavs = [
                cst.tile([128, 512], F16, tag=f"wav{et}", name=f"wav{et}")
                for et in range(NET)
            ]
            xts = [
                cst.tile([128, S2], F16, tag=f"xt{et}", name=f"xt{et}")
                for et in range(NET)
            ]
            wps = [
                cst.tile([128, E], F16, tag=f"wp{et}", name=f"wp{et}")
                for et in range(NET)
            ]
            # fp8 qk-gen operands first (the lead-in only needs these)
            nc.sync.dma_start(out=wa8_t, in_=wa8[:, :])
            nc.sync.dma_start(out=xt8_t[:, 0:2048], in_=xt8[:, 0:2048])
            nc.sync.dma_start(out=xt8_t[:, 2048:4096], in_=xt8[:, 2048:4096])
            bqk_t = cst.tile([128, 8], F32)
            nc.sync.dma_start(out=bqk_t, in_=bqk[:, :])
            mask2_t = cst.tile([128, 256], F16)
            nc.sync.dma_start(out=mask2_t, in_=mask2[:, :])
            for et in range(NET):
                nc.sync.dma_start(
                    out=xts[et][:, 0:1024],
                    in_=xt[128 * et : 128 * (et + 1), 0:1024],
                )
                nc.sync.dma_start(
                    out=wavs[et],
                    in_=wav[128 * et : 128 * (et + 1), :],
                )
            nc.sync.dma_start(out=xt8_t[:, 4096:6144], in_=xt8[:, 4096:6144])
            nc.sync.dma_start(out=xt8_t[:, 6144:8192], in_=xt8[:, 6144:8192])
            for et in range(NET):
                nc.sync.dma_start(
                    out=xts[et][:, 1024:2048],
                    in_=xt[128 * et : 128 * (et + 1), 1024:2048],
                )
            for et in range(NET):
                nc.sync.dma_start(
                    out=wps[et], in_=wp[128 * et : 128 * (et + 1), :]
                )
            bcol_t = cst.tile([128, 4], F32)
            nc.sync.dma_start(out=bcol_t, in_=bcol[:, :])
            # persistent v_ext tiles: [128 k-rows, 8h * (64 v | 64 ones)];
            # ones columns are constant, v columns written by qkv v-units
            vxs = {}
            for b in range(BPC):
                for st in range(NQT):
                    v_ = vxp.tile(
                        [128, 8 * 128], F16, tag=f"vx{b}_{st}", name=f"vx{b}_{st}"
                    )
                    nc.sync.dma_start(out=v_, in_=vinit[:, :])
                    vxs[(b, st)] = v_

            qkts = {
                (b, m): qkp.tile(
                    [128, S], F16, tag=f"qk{b}_{m}", name=f"qkT{b}_{m}"
                )
                for b in range(BPC)
                for m in range(8)
            }
            ots = {
                (b, i): otp.tile(
                    [128, S], F16, tag=f"ot{b}_{i}", name=f"oT{b}_{i}"
                )
                for b in range(BPC)
                for i in range(4)
            }

            # ---- work units (emitted inline between attention steps so
            # dense PE work fills Scalar-engine stalls) ----
            emitted = set()

            wa8_4d = wa8_t[:, :].rearrange("p (e g j) -> p e g j", e=2, g=2)
            xt8_5d = xt8_t[:, :].rearrange(
                "p (c e g s) -> p c e g s", c=4, e=2, g=2
            )

            def qk_unit(b, m, sc, lead=False):
                """qkT[m] seq-chunk sc for batch b (+ bias).  Lead-in
                units allocate from the (idle at start) scores pool so the
                four units pipeline instead of serializing through the
                single pq bank, and evacuate on Scalar only."""
                pool = ps if lead else pqp
                tag = "ps" if lead else "pq"
                pq0 = pool.tile(
                    [128, 512], F32, tag=tag, name=f"pq_{b}_{m}_{sc}"
                )
                for es in range(2):
                    nc.tensor.matmul(
                        pq0,
                        wa8_4d[:, es, :, 128 * m : 128 * (m + 1)],
                        xt8_5d[:, 2 * b + sc, es, :, :],
                        start=(es == 0), stop=(es == 1),
                        perf_mode=DR,
                    )
                # alternate evacuation engine so the single pq psum buffer
                # recycles quickly (Identity+bias is in the exp table set,
                # so Scalar use causes no table thrash)
                if sc == 0 or lead:
                    nc.scalar.activation(
                        qkts[(b, m)][:, 512 * sc : 512 * (sc + 1)], pq0,
                        func=mybir.ActivationFunctionType.Identity,
                        bias=bqk_t[:, m : m + 1],
                    )
                else:
                    nc.vector.tensor_scalar_add(
                        qkts[(b, m)][:, 512:1024], pq0, bqk_t[:, m : m + 1]
                    )
                emitted.add(("qk", b, m, sc))

            def v_unit(b, st):
                """v rows for seq-tile st of batch b -> v_ext v-columns."""
                pv_ = pqp.tile([128, 512], F32, tag="pq", name=f"pv_{b}_{st}")
                s0 = b * S + 128 * st
                for et in range(NET):
                    nc.tensor.matmul(
                        pv_,
                        xts[et][:, s0 : s0 + 128],
                        wavs[et][:, :],
                        start=(et == 0),
                        stop=(et == NET - 1),
                    )
                dst = vxs[(b, st)][:, :].rearrange("p (h c) -> p h c", h=H)
                dst = dst[:, :, 0:64]
                src = pv_[:, :].rearrange("p (h c) -> p h c", h=H)
                nc.vector.tensor_copy(dst, src)
                emitted.add(("v", b, st))

            def proj_unit(b, eb, sc, alt_pool=False):
                """output e-rows [128*eb:...] for seq-chunk sc of batch b
                (out is [E, S2]).  Tail units alternate psum pools so they
                pipeline instead of serializing through one bank."""
                pool = ps if alt_pool else pqp
                tag = "ps" if alt_pool else "pq"
                pp_ = pool.tile(
                    [128, 512], F32, tag=tag, name=f"pp_{b}_{eb}_{sc}"
                )
                for i in range(4):
                    nc.tensor.matmul(
                        pp_,
                        wps[i][:, 128 * eb : 128 * (eb + 1)],
                        ots[(b, i)][:, 512 * sc : 512 * (sc + 1)],
                        start=(i == 0), stop=(i == 3),
                    )
                ob_ = obp.tile(
                    [128, 512], F16, tag="ob", name=f"ob_{b}_{eb}_{sc}"
                )
                if sc == 0:
                    nc.scalar.activation(
                        ob_, pp_,
                        func=mybir.ActivationFunctionType.Identity,
                        bias=bcol_t[:, eb : eb + 1],
                    )
                else:
                    nc.vector.tensor_scalar_add(
                        ob_, pp_, bcol_t[:, eb : eb + 1]
                    )
                s0 = b * S + 512 * sc
                nc.sync.dma_start(
                    out=out[128 * eb : 128 * (eb + 1), s0 : s0 + 512],
                    in_=ob_,
                )

            def attn_pair(b, i, bg, slots=None):
                """Causal attention for heads (2i, 2i+1) of batch b.
                bg: list of background thunks consumed inside the j-loop."""
                for mm in (i, 4 + i):
                    for sc in range(2):
                        assert ("qk", b, mm, sc) in emitted
                kt = qkts[(b, 4 + i)]
                qt = qkts[(b, i)]
                # per-j exp tile holding BOTH heads: head hh at cols
                # hh*1024 + q (one mask instruction covers the pair)
                exs = {
                    j: exp_pool.tile(
                        [128, 2 * S], F16, tag="ex", name=f"ex_{b}_{i}_{j}"
                    )
                    for j in range(NQT)
                }

                bg = list(bg)
                nbg = len(bg)
                if slots is None:
                    # distribute bg thunks across the 8 j-iterations
                    slots = [nbg // NQT + (1 if j < nbg % NQT else 0)
                             for j in range(NQT)]
                slots = list(slots)
                assert sum(slots) == nbg
                bi = 0

                # qc1 accumulators live across the whole j-loop so their
                # matmuls can issue right after each k-block's exp (no
                # j==7 burst waiting on the Scalar engine)
                po1 = {
                    hh: pop.tile(
                        [128, 512], F32, tag="po", name=f"po1_{b}_{i}_{hh}"
                    )
                    for hh in range(2)
                }

                def attnv_qc1_step(hh, jj):
                    h = 2 * i + hh
                    c0 = max(512, 128 * jj)
                    nc.tensor.matmul(
                        po1[hh][:, c0 - 512 : 512],
                        vxs[(b, jj)][:, 128 * h : 128 * (h + 1)],
                        exs[jj][:, 1024 * hh + c0 : 1024 * hh + 1024],
                        start=(jj == 0),
                        stop=(jj == NQT - 1),
                    )

                def attn_pass(hh, qc):
                    h = 2 * i + hh
                    if qc == 1:
                        po_ = po1[hh]
                    else:
                        po_ = pop.tile(
                            [128, 512], F32, tag="po",
                            name=f"po_{b}_{i}_{hh}_{qc}",
                        )
                        for jj in range(4):
                            c0 = 128 * jj
                            nc.tensor.matmul(
                                po_[:, c0:512],
                                vxs[(b, jj)][:, 128 * h : 128 * (h + 1)],
                                exs[jj][:, 1024 * hh + c0 : 1024 * hh + 512],
                                start=(jj == 0),
                                stop=(jj == 3),
                            )
                    # 1/denominator via bit-trick seed + one Newton step
                    # (rel err ~2.6e-3, HW-verified); denominator copies sit
                    # in po rows 64-127 thanks to the ones-extended v_ext
                    sfx = f"{b}_{i}_{hh}_{qc}"
                    s_ = nrp.tile([64, 512], F32, tag="nr", name=f"nrs_{sfx}")
                    t_ = nrp.tile([64, 512], F32, tag="nr", name=f"nrt_{sfx}")
                    u_ = nrp.tile([64, 512], F32, tag="nr", name=f"nru_{sfx}")
                    nc.vector.tensor_scalar(
                        out=s_[:, :].bitcast(I32),
                        in0=po_[64:128, :].bitcast(I32),
                        scalar1=RECIP_MAGIC,
                        scalar2=-1.0,
                        op0=mybir.AluOpType.subtract,
                        op1=mybir.AluOpType.mult,
                    )
                    nc.vector.tensor_mul(t_, po_[64:128, :], s_)
                    # (GpSimd cannot take this step: [64,512] f32 ops on the
                    # Q7s measured ~4-8x slower and serialized the kernel)
                    nc.vector.scalar_tensor_tensor(
                        out=u_, in0=t_, scalar=2.0, in1=s_,
                        op0=mybir.AluOpType.subtract,
                        op1=mybir.AluOpType.mult,
                    )
                    nc.vector.scalar_tensor_tensor(
                        out=ots[(b, i)][64 * hh : 64 * hh + 64,
                                        512 * qc : 512 * (qc + 1)],
                        in0=po_[0:64, :], scalar=-1.0, in1=u_,
                        op0=mybir.AluOpType.mult,
                        op1=mybir.AluOpType.mult,
                    )

                for j in range(NQT):
                    nqc = sum(
                        1 for qc in range(2)
                        if max(512 * qc, 128 * j) < 512 * (qc + 1)
                    )
                    for qc in range(2):
                        c0 = max(512 * qc, 128 * j)
                        c1 = 512 * (qc + 1)
                        if c0 >= c1:
                            continue
                        w = c1 - c0
                        pA = ps.tile(
                            [128, 512], F32, tag="ps", name=f"sA_{b}_{i}_{j}_{qc}"
                        )
                        pB = ps.tile(
                            [128, 512], F32, tag="ps", name=f"sB_{b}_{i}_{j}_{qc}"
                        )
                        nc.tensor.matmul(
                            pA[:, 0:w],
                            kt[0:64, 128 * j : 128 * (j + 1)],
                            qt[0:64, c0:c1],
                            start=True,
                            stop=True,
                        )
                        nc.tensor.matmul(
                            pB[:, 0:w],
                            kt[64:128, 128 * j : 128 * (j + 1)],
                            qt[64:128, c0:c1],
                            start=True,
                            stop=True,
                        )
                        nc.scalar.activation(
                            exs[j][:, c0:c1], pA[:, 0:w],
                            func=EXP, scale=EXP_SCALE,
                        )
                        nc.scalar.activation(
                            exs[j][:, 1024 + c0 : 1024 + c1],
                            pB[:, 0:w],
                            func=EXP, scale=EXP_SCALE,
                        )
                        # keep the PE fed between the two psum-chunk
                        # rounds (the ps pool is fully occupied here)
                        if nqc == 2 and qc == 0 and bi < nbg and slots[j] > 0:
                            bg[bi]()
                            bi += 1
                            slots[j] -= 1
                    # causal mask on the diagonal 128-col blocks (GpSimd:
                    # SBUF-only f16 work; 2D APs only - 3D APs are ~6x
                    # slower on the Q7s)
                    nc.gpsimd.tensor_mul(
                        exs[j][:, 128 * j : 128 * (j + 1)],
                        exs[j][:, 128 * j : 128 * (j + 1)],
                        mask2_t[:, 0:128],
                    )
                    nc.gpsimd.tensor_mul(
                        exs[j][:, 1024 + 128 * j : 1024 + 128 * (j + 1)],
                        exs[j][:, 1024 + 128 * j : 1024 + 128 * (j + 1)],
                        mask2_t[:, 0:128],
                    )
                    # stream this k-block into the qc1 accumulators now
                    # that the diagonal is masked
                    attnv_qc1_step(0, j)
                    attnv_qc1_step(1, j)
                    for _ in range(slots[j]):
                        bg[bi]()
                        bi += 1
                    if j == 3:
                        attn_pass(0, 0)
                        attn_pass(1, 0)
                    if j == NQT - 1:
                        attn_pass(0, 1)
                        attn_pass(1, 1)
                assert bi == nbg

            # ---- schedule ----
            # lead-in: qk tiles for pair (0,0)
            for m in (0, 4):
                for sc in range(2):
                    qk_unit(0, m, sc, lead=True)

            def mkqk(b, m, sc):
                return lambda: qk_unit(b, m, sc)

            def mkv(b, st):
                return lambda: v_unit(b, st)

            def mkp(b, eb, sc):
                return lambda: proj_unit(b, eb, sc)

            def qks(b, m):
                return [mkqk(b, m, 0), mkqk(b, m, 1)]

            bg_for_pair = {
                (0, 0): [mkv(0, 0), mkv(0, 1), mkv(0, 2), mkv(0, 3),
                         mkqk(0, 1, 0), mkqk(0, 5, 0),
                         mkv(0, 4), mkv(0, 5), mkv(0, 6), mkv(0, 7),
                         mkqk(0, 1, 1), mkqk(0, 5, 1)],
                (0, 1): qks(0, 2) + qks(0, 6),
                (0, 2): qks(0, 3) + qks(0, 7),
                (0, 3): qks(1, 0) + qks(1, 4),
                (1, 0): [mkv(1, 0), mkv(1, 1), mkv(1, 2), mkv(1, 3),
                         mkqk(1, 1, 0), mkqk(1, 5, 0),
                         mkv(1, 4), mkv(1, 5), mkv(1, 6), mkv(1, 7),
                         mkqk(1, 1, 1), mkqk(1, 5, 1)],
                (1, 1): qks(1, 2) + qks(1, 6),
                (1, 2): qks(1, 3) + qks(1, 7)
                + [mkp(0, 0, 0), mkp(0, 0, 1), mkp(0, 1, 0), mkp(0, 1, 1)],
                # proj(1, eb, sc0) needs only the qc0 norms of batch 1 -
                # its own pair's land at j==3, so slots 4..7 are safe
                (1, 3): [mkp(0, 2, 0), mkp(0, 2, 1),
                         mkp(0, 3, 0), mkp(0, 3, 1),
                         mkp(1, 0, 0), mkp(1, 1, 0),
                         mkp(1, 2, 0), mkp(1, 3, 0)],
            }
            slots_override = {(1, 3): [1, 1, 1, 1, 1, 1, 1, 1]}
            for b in range(BPC):
                for i in range(4):
                    attn_pair(
                        b, i, bg_for_pair[(b, i)],
                        slots=slots_override.get((b, i)),
                    )
            # tail: batch-1 projection second halves (alternating psum
            # pools so evacuations overlap the next unit's matmuls)
            for eb in range(4):
                proj_unit(1, eb, 1, alt_pool=(eb % 2 == 1))
    if salt:
        # unique NoOp name changes the BIR bytes -> defeats NEFF caching
        # when only compiler flags change
        nop = mybir.InstNoOp(name=f"salt_{salt}", ins=[], outs=[])
        nop.engine = mybir.EngineType.SP
        nop.sync_info = mybir.SyncInfo(on_wait=[], on_update=[])
        nc.m.functions[0].blocks[0].instructions.append(nop)
    if dedupe:
        _dedupe_ldweights(nc)
    if split_waits:
        _split_multi_waits(nc)
    return nc


# ---------------------------------------------------------------------------
# Host orchestration
# ---------------------------------------------------------------------------

_CACHE = {}


def _get(name, builder):
    if name not in _CACHE:
        _CACHE[name] = builder()
    return _CACHE[name]


def _run_with_retry(nc, in_maps, trace=False, tries=3):
    import time as _time

    last = None
    for attempt in range(tries):
        try:
            return run_bass_kernel_spmd(
                nc, in_maps, core_ids=list(range(NCORES)), trace=trace
            )
        except Exception as e:  # transient NRT_EXEC_UNIT_UNRECOVERABLE etc.
            last = e
            _time.sleep(2.0 * (attempt + 1))
    raise last


def _silu(v):
    return v / (1.0 + np.exp(-v))


def kernel(
    time_embed,
    x,
    lin1_w,
    lin1_b,
    lin2_w,
    lin2_b,
    fW_attn_w,
    fW_attn_b,
    fb_attn,
    fW_proj_w,
    fW_proj_b,
    fb_proj,
    _trace=False,
    _times=None,
):
    f64 = np.float64
    # ---- host: time-embedding MLP ----
    t1 = _silu(time_embed.astype(f64) @ lin1_w.astype(f64) + lin1_b.astype(f64))
    t = t1 @ lin2_w.astype(f64) + lin2_b.astype(f64)   # [128]
    t32 = t.astype(np.float32)

    # ---- host: hypernetwork weights (the host reads fW to cast anyway;
    # this matvec is 0.4% of the problem's FLOPs) ----
    Wa = t32 @ fW_attn_w.reshape(TEMBED, E * J3)
    Wa = Wa.reshape(E, J3) + fW_attn_b.reshape(E, J3)
    Wp = t32 @ fW_proj_w.reshape(TEMBED, E * E)
    Wp = Wp.reshape(E, E) + fW_proj_b.reshape(E, E)
    Wa[:, :512] *= 0.125  # fold 1/sqrt(D) into q columns

    # ---- host: fp8 qk-gen stationary: [128, (es, g, 1024 qk-cols)] ----
    fp8np = mybir.dt.np(FP8)
    Wa_qk = (Wa[:, :1024] * WA_SCALE).astype(fp8np)
    wa8_in = np.ascontiguousarray(
        Wa_qk.reshape(2, 2, 128, 1024).transpose(2, 0, 1, 3).reshape(128, 4096)
    )

    # ---- host: biases ----
    b_attn = (t @ fb_attn.astype(f64).reshape(TEMBED, J3)).astype(np.float32)
    bqk_host = b_attn[:1024].copy()
    bqk_host[:512] *= 0.125
    bqk_host *= WA_SCALE  # qkT tiles are stored x32
    bqk_in = np.ascontiguousarray(bqk_host.reshape(8, 128).T)
    b_v = b_attn[1024:]
    b_proj = (t @ fb_proj.astype(f64)).astype(np.float32)
    brow = (b_v.astype(f64) @ Wp.astype(f64) + b_proj).astype(np.float32)
    bcol_in = np.ascontiguousarray(brow.reshape(4, 128).T)
    mask1 = np.triu(np.ones((128, 128), dtype=np.float16))
    mask2_in = np.ascontiguousarray(np.concatenate([mask1, mask1], axis=1))
    vinit_in = np.zeros((128, S), dtype=np.float16)
    for h in range(H):
        vinit_in[:, 128 * h + 64 : 128 * (h + 1)] = 1.0
    Wav16 = np.ascontiguousarray(Wa[:, 1024:1536].astype(np.float16))
    Wp16 = Wp.astype(np.float16)

    # ---- launch: attention ----
    nc_attn = _get("attn", build_attn)
    in_maps = []
    for c in range(NCORES):
        xt_c = np.ascontiguousarray(
            x[BPC * c : BPC * (c + 1)].reshape(S2, E).T
        )
        xt8_c = np.ascontiguousarray(
            xt_c.astype(fp8np)
            .reshape(2, 2, 128, 4, 512)     # [es][g][p][sc][sl]
            .transpose(2, 3, 0, 1, 4)       # [p][sc][es][g][sl]
            .reshape(128, 8192)
        )
        in_maps.append(
            {
                "wa8": wa8_in,
                "xt8": xt8_c,
                "xt": xt_c.astype(np.float16),
                "wav": Wav16,
                "wp": Wp16,
                "bqk": bqk_in,
                "bcol": bcol_in,
                "mask2": mask2_in,
                "vinit": vinit_in,
            }
        )
    res = _run_with_retry(nc_attn, in_maps, trace=_trace)
    if _times is not None:
        _times.append(res.exec_time_ns)

    out = np.empty((B, S, E), dtype=np.float32)
    for c in range(NCORES):
        out[BPC * c : BPC * (c + 1)] = (
            res.results[c]["out"].astype(np.float32).T.reshape(BPC, S, E)
        )
    return out

